# revision 14
# baseline (speedup 1.0000x reference)
"""Chamfer distance kernel for Trainium2 (8 NeuronCores).

Problem: pred/target [4, 8192, 3] f32 -> scalar
  mean_b( mean_m min_n ||p_bm - q_bn||^2 + mean_n min_m ||p_bm - q_bn||^2 )

Strategy (one "side" per core; 4 batches x 2 directions = 8 cores):
  Each core owns one (batch, direction) pair. Both clouds are sorted on the
  host along one coordinate axis; each m-tile of 128 "own" points only
  computes distances to a static window of WP=512 "other" points centered
  at the matching rank (rank-locality of nearest neighbors in a sorted
  cloud). The result is certified exactly on the host: a point whose
  windowed min distance is smaller than the squared distance from its sort
  key to the window's edge key provably has its true NN inside the window;
  the ~70 uncertified points per cloud get an exact O(N) host recheck.
  This cuts device work 16x versus the full 8192x8192 sweep while keeping
  the result exact up to fp16 rounding of the certified mins.

  Distances are produced on the TensorEngine as K=8 matmuls using the
  identity ||p-q||^2 = -2 p.q + ||p||^2 + ||q||^2:
      lhsT rows: [-2x, -2y, -2z, n_hi, n_lo, 1, 1, 0]   (own points)
      rhs  rows: [ x,   y,  z,  1,    1,  n_hi, n_lo, 0] (other points)
  Inputs are fp16; norms are split hi/lo into two fp16 values so the norm
  contribution keeps ~2^-22 precision; fp16 products are exact in the fp32
  PSUM accumulation. Host prep replicates lhsT/rhs to partition offsets
  0/32/64/96 so four matmuls pack into disjoint 32-row groups via
  tile_position (full-width [128, n] DMAs; narrow [8, n] transfers only
  engage 8 of 128 SBUF lanes and pace the whole pipeline). Input DMA is
  chunked in 1024-column pieces so the first quads start ~1us in. One
  [128, 2048] PSUM tile holds FOUR m-tiles' windows (one bank-aligned
  512-wide out each).

  Drain (the DVE can read at most ONE PSUM operand per instruction):
  3 of every 8 m-tiles are min-reduced straight out of PSUM by the DVE
  (tensor_reduce, fp32); the other 5 are copied to fp16 SBUF by ScalarE
  and folded 512->64 at fp16 2x + min-reduced by the DVE, batched per
  group of 8 m-tiles. This balances ScalarE and VectorE at ~0.45us per
  m-tile each.
"""

import numpy as np

import concourse.bacc as bacc
import concourse.mybir as mybir
import concourse.tile as tile
from concourse import bass_utils

P = 128          # partitions / m-tile size
NPTS = 8192      # points per cloud
B = 4            # batch
K = 8            # matmul contraction (padded)
MT = NPTS // P   # 64 m-tiles
WP = 512         # window columns per m-tile (one PSUM bank)
GRP = 8          # m-tiles per drain-balancing group (2 PSUM quads)
DIRECT = (1, 3, 5)            # j-in-group drained by DVE straight from PSUM
ASSIST = (0, 2, 4, 6, 7)      # j-in-group staged by ScalarE + ladder
ND = len(DIRECT)
NA = len(ASSIST)
DCHUNK = 1024    # input DMA chunk columns (pipelines input vs compute)

F16 = mybir.dt.float16
F32 = mybir.dt.float32
MIN = mybir.AluOpType.min
AXX = None  # set below


def _win_start(t):
    return min(max(t * P + P // 2 - WP // 2, 0), NPTS - WP)


def _build_nc():
    nc = bacc.Bacc(
        "TRN2", target_bir_lowering=False, debug=False, num_devices=8
    )
    lhsT_d = nc.dram_tensor("lhsT", [P, NPTS], F16, kind="ExternalInput")
    rhs_d = nc.dram_tensor("rhs", [P, NPTS], F16, kind="ExternalInput")
    mind_d = nc.dram_tensor("mind", [P, ND * (MT // GRP)], F32,
                            kind="ExternalOutput")
    mina_d = nc.dram_tensor("mina", [P, NA * (MT // GRP)], F32,
                            kind="ExternalOutput")

    with tile.TileContext(nc) as tc:
        with (
            tc.tile_pool(name="const", bufs=1) as const,
            tc.tile_pool(name="psum", bufs=2, space="PSUM") as psum,
            tc.tile_pool(name="xpool", bufs=3) as xpool,
            tc.tile_pool(name="wpool", bufs=2) as wpool,
        ):
            lt4 = const.tile([P, NPTS], F16)
            rt4 = const.tile([P, NPTS], F16)
            res_d = const.tile([P, ND * (MT // GRP)], F32)
            res_a = const.tile([P, NA * (MT // GRP)], F32)
            # full-width chunked input DMA. Each dma_start costs ~640ns of
            # serialized issue time on its sequencer, so: few chunks, sized
            # small-first (fast pipeline start) to large-last, with the two
            # streams issued from different engines (rhs on SP, lhsT on the
            # otherwise-idle GpSimd SWDGE path).
            for c0, w in ((0, 1024), (1024, 1024), (2048, 2048),
                          (4096, 4096)):
                nc.sync.dma_start(
                    rt4[:, c0:c0 + w], rhs_d.ap()[:, c0:c0 + w]
                )
            for c0, w in ((0, 1024), (1024, 1024), (2048, 2048),
                          (4096, 4096)):
                nc.gpsimd.dma_start(
                    lt4[:, c0:c0 + w], lhsT_d.ap()[:, c0:c0 + w]
                )

            for g in range(MT // GRP):
                xb = xpool.tile([P, NA, WP], F16, tag="xb")
                for q in range(2):
                    ps = psum.tile([P, 4 * WP], F32, tag="ps")
                    for u in range(4):
                        t = g * GRP + 4 * q + u
                        S = _win_start(t)
                        nc.tensor.matmul(
                            ps[:, u * WP:(u + 1) * WP],
                            lt4[32 * u:32 * u + K, t * P:(t + 1) * P],
                            rt4[32 * u:32 * u + K, S:S + WP],
                            start=True,
                            stop=True,
                            tile_position=(32 * u, 0),
                        )
                    for u in range(4):
                        j = 4 * q + u
                        t = g * GRP + j
                        sl = ps[:, u * WP:(u + 1) * WP]
                        if j in DIRECT:
                            di = g * ND + DIRECT.index(j)
                            nc.vector.tensor_reduce(
                                res_d[:, di:di + 1], sl,
                                axis=mybir.AxisListType.X, op=MIN,
                            )
                        else:
                            nc.scalar.copy(xb[:, ASSIST.index(j), :], sl)
                # batched tail: fold 512->64 at fp16 2x, min-reduce 64->1
                v1 = wpool.tile([P, NA, WP // 2], F16, tag="v1")
                nc.vector.tensor_tensor(
                    v1[:], xb[:, :, :WP // 2], xb[:, :, WP // 2:], op=MIN
                )
                v2 = wpool.tile([P, NA, WP // 4], F16, tag="v2")
                nc.vector.tensor_tensor(
                    v2[:], v1[:, :, :WP // 4], v1[:, :, WP // 4:], op=MIN
                )
                v3 = wpool.tile([P, NA, WP // 8], F16, tag="v3")
                nc.vector.tensor_tensor(
                    v3[:], v2[:, :, :WP // 8], v2[:, :, WP // 8:], op=MIN
                )
                nc.vector.tensor_reduce(
                    res_a[:, g * NA:(g + 1) * NA], v3[:],
                    axis=mybir.AxisListType.X, op=MIN,
                )

            nc.sync.dma_start(mind_d.ap(), res_d[:])
            nc.sync.dma_start(mina_d.ap(), res_a[:])

    nc.compile()
    return nc


_NC_CACHE = []


def _get_nc():
    if not _NC_CACHE:
        _NC_CACHE.append(_build_nc())
    return _NC_CACHE[0]


def _prep_side(own, other):
    """Build lhsT [128, N] and rhs [128, N] fp16 with the K=8 row content
    replicated at partition offsets 0/32/64/96 for row-group packing."""
    o16 = own.astype(np.float16)
    t16 = other.astype(np.float16)
    o32 = o16.astype(np.float32)
    t32 = t16.astype(np.float32)
    on = (o32 * o32).sum(-1)       # fp32 norms of the fp16-rounded points
    tn = (t32 * t32).sum(-1)
    on_hi = on.astype(np.float16)
    on_lo = (on - on_hi.astype(np.float32)).astype(np.float16)
    tn_hi = tn.astype(np.float16)
    tn_lo = (tn - tn_hi.astype(np.float32)).astype(np.float16)

    n = own.shape[0]
    lhsT = np.zeros((K, n), np.float16)
    lhsT[0:3] = (-2.0 * o32).astype(np.float16).T
    lhsT[3] = on_hi
    lhsT[4] = on_lo
    lhsT[5] = 1.0
    lhsT[6] = 1.0
    rhs = np.zeros((K, n), np.float16)
    rhs[0:3] = t16.T
    rhs[3] = 1.0
    rhs[4] = 1.0
    rhs[5] = tn_hi
    rhs[6] = tn_lo

    lhsT4 = np.zeros((P, n), np.float16)
    rhs4 = np.zeros((P, n), np.float16)
    for g in range(4):
        lhsT4[32 * g:32 * g + K] = lhsT
        rhs4[32 * g:32 * g + K] = rhs
    return lhsT4, rhs4


def _sides(pred, target):
    """Per-core (own_sorted, other_sorted, axis) for the 8 (batch,
    direction) pairs, with both clouds sorted along the batch's
    max-variance axis."""
    pred = np.asarray(pred, dtype=np.float32)
    target = np.asarray(target, dtype=np.float32)
    sides = []
    for b in range(B):
        axis = int(np.argmax(pred[b].var(0) + target[b].var(0)))
        for d in range(2):
            own, other = (
                (pred[b], target[b]) if d == 0 else (target[b], pred[b])
            )
            so = np.argsort(own[:, axis], kind="stable")
            st = np.argsort(other[:, axis], kind="stable")
            sides.append((own[so], other[st], axis))
    return sides


def _in_maps_for(pred, target):
    in_maps = []
    for own_s, oth_s, _axis in _sides(pred, target):
        lhsT, rhs = _prep_side(own_s, oth_s)
        in_maps.append({"lhsT": lhsT, "rhs": rhs})
    return in_maps


def _assemble_mins(core_res):
    """[NPTS] windowed min per sorted-own point from the two outputs."""
    res_d = core_res["mind"].astype(np.float64)
    res_a = core_res["mina"].astype(np.float64)
    w = np.empty(NPTS)
    for g in range(MT // GRP):
        for j in range(GRP):
            t = g * GRP + j
            if j in DIRECT:
                col = res_d[:, g * ND + DIRECT.index(j)]
            else:
                col = res_a[:, g * NA + ASSIST.index(j)]
            w[t * P:(t + 1) * P] = col
    return w


def kernel(pred, target):
    sides = _sides(pred, target)
    in_maps = []
    for own_s, oth_s, _axis in sides:
        lhsT, rhs = _prep_side(own_s, oth_s)
        in_maps.append({"lhsT": lhsT, "rhs": rhs})
    nc = _get_nc()
    r = bass_utils.run_bass_kernel_spmd(nc, in_maps, core_ids=list(range(8)))

    total = 0.0
    for core_res, (own_s, oth_s, axis) in zip(r.results, sides):
        w = _assemble_mins(core_res)
        # certification: window covers the true NN unless the windowed min
        # exceeds the squared key-distance to the window edge
        okey = oth_s[:, axis].astype(np.float64)
        own_key = own_s[:, axis].astype(np.float64)
        g = np.empty(NPTS)
        for t in range(MT):
            S = _win_start(t)
            ok = own_key[t * P:(t + 1) * P]
            gl = np.inf if S == 0 else ok - okey[S]
            gr = np.inf if S + WP == NPTS else okey[S + WP - 1] - ok
            g[t * P:(t + 1) * P] = np.minimum(gl, gr)
        uncert = np.nonzero(w > 0.98 * g * g)[0]
        if uncert.size:
            d = own_s[uncert, None, :].astype(np.float64) - oth_s[None, :, :]
            w[uncert] = (d * d).sum(-1).min(1)
        total += w.mean()
    return np.array(total / B, dtype=np.float32)


# revision 15
# speedup vs baseline: 1.0245x; 1.0245x over previous
"""Chamfer distance kernel for Trainium2 (8 NeuronCores).

Problem: pred/target [4, 8192, 3] f32 -> scalar
  mean_b( mean_m min_n ||p_bm - q_bn||^2 + mean_n min_m ||p_bm - q_bn||^2 )

Strategy (one "side" per core; 4 batches x 2 directions = 8 cores):
  Each core owns one (batch, direction) pair. Both clouds are sorted on the
  host along one coordinate axis; each m-tile of 128 "own" points only
  computes distances to a static window of WP=512 "other" points centered
  at the matching rank (rank-locality of nearest neighbors in a sorted
  cloud). The result is certified exactly on the host: a point whose
  windowed min distance is smaller than the squared distance from its sort
  key to the window's edge key provably has its true NN inside the window;
  the ~70 uncertified points per cloud get an exact O(N) host recheck.
  This cuts device work 16x versus the full 8192x8192 sweep while keeping
  the result exact up to fp16 rounding of the certified mins.

  Distances are produced on the TensorEngine as K=8 matmuls using the
  identity ||p-q||^2 = -2 p.q + ||p||^2 + ||q||^2:
      lhsT rows: [-2x, -2y, -2z, n_hi, n_lo, 1, 1, 0]   (own points)
      rhs  rows: [ x,   y,  z,  1,    1,  n_hi, n_lo, 0] (other points)
  Inputs are fp16; norms are split hi/lo into two fp16 values so the norm
  contribution keeps ~2^-22 precision; fp16 products are exact in the fp32
  PSUM accumulation. Host prep replicates lhsT/rhs to partition offsets
  0/32/64/96 so four matmuls pack into disjoint 32-row groups via
  tile_position (full-width [128, n] DMAs; narrow [8, n] transfers only
  engage 8 of 128 SBUF lanes and pace the whole pipeline). Input DMA is
  chunked in 1024-column pieces so the first quads start ~1us in. One
  [128, 2048] PSUM tile holds FOUR m-tiles' windows (one bank-aligned
  512-wide out each).

  Drain (the DVE can read at most ONE PSUM operand per instruction):
  3 of every 8 m-tiles are min-reduced straight out of PSUM by the DVE
  (tensor_reduce, fp32); the other 5 are copied to fp16 SBUF by ScalarE
  and folded 512->64 at fp16 2x + min-reduced by the DVE, batched per
  group of 8 m-tiles. This balances ScalarE and VectorE at ~0.45us per
  m-tile each.
"""

import numpy as np

import concourse.bacc as bacc
import concourse.mybir as mybir
import concourse.tile as tile
from concourse import bass_utils

P = 128          # partitions / m-tile size
NPTS = 8192      # points per cloud
B = 4            # batch
K = 8            # matmul contraction (padded)
MT = NPTS // P   # 64 m-tiles
WP = 512         # window columns per m-tile (one PSUM bank)
GRP = 8          # m-tiles per drain-balancing group (2 PSUM quads)
DIRECT = (1, 3, 5)            # j-in-group drained by DVE straight from PSUM
ASSIST = (0, 2, 4, 6, 7)      # j-in-group staged by ScalarE + ladder
ND = len(DIRECT)
NA = len(ASSIST)
DCHUNK = 1024    # input DMA chunk columns (pipelines input vs compute)

F16 = mybir.dt.float16
F32 = mybir.dt.float32
MIN = mybir.AluOpType.min
AXX = None  # set below


def _win_start(t):
    return min(max(t * P + P // 2 - WP // 2, 0), NPTS - WP)


def _build_nc():
    nc = bacc.Bacc(
        "TRN2", target_bir_lowering=False, debug=False, num_devices=8
    )
    lhsT_d = nc.dram_tensor("lhsT", [P, NPTS], F16, kind="ExternalInput")
    rhs_d = nc.dram_tensor("rhs", [P, NPTS], F16, kind="ExternalInput")
    mind_d = nc.dram_tensor("mind", [P, ND * (MT // GRP)], F32,
                            kind="ExternalOutput")
    mina_d = nc.dram_tensor("mina", [P, NA * (MT // GRP)], F32,
                            kind="ExternalOutput")

    with tile.TileContext(nc) as tc:
        with (
            tc.tile_pool(name="const", bufs=1) as const,
            tc.tile_pool(name="psum", bufs=2, space="PSUM") as psum,
            tc.tile_pool(name="xpool", bufs=3) as xpool,
            tc.tile_pool(name="wpool", bufs=2) as wpool,
        ):
            lt4 = const.tile([P, NPTS], F16)
            rt4 = const.tile([P, NPTS], F16)
            res_d = const.tile([P, ND * (MT // GRP)], F32)
            res_a = const.tile([P, NA * (MT // GRP)], F32)
            # full-width chunked input DMA. Each dma_start costs ~640ns of
            # serialized issue time on its sequencer, so: few chunks, sized
            # small-first (fast pipeline start) to large-last, with the two
            # streams issued from different engines (rhs on SP, lhsT on the
            # otherwise-idle GpSimd SWDGE path).
            for c0, w in ((0, 1024), (1024, 1024), (2048, 2048),
                          (4096, 4096)):
                nc.sync.dma_start(
                    rt4[:, c0:c0 + w], rhs_d.ap()[:, c0:c0 + w]
                )
                nc.sync.dma_start(
                    lt4[:, c0:c0 + w], lhsT_d.ap()[:, c0:c0 + w]
                )

            for g in range(MT // GRP):
                xb = xpool.tile([P, NA, WP], F16, tag="xb")
                for q in range(2):
                    ps = psum.tile([P, 4 * WP], F32, tag="ps")
                    for u in range(4):
                        t = g * GRP + 4 * q + u
                        S = _win_start(t)
                        nc.tensor.matmul(
                            ps[:, u * WP:(u + 1) * WP],
                            lt4[32 * u:32 * u + K, t * P:(t + 1) * P],
                            rt4[32 * u:32 * u + K, S:S + WP],
                            start=True,
                            stop=True,
                            tile_position=(32 * u, 0),
                        )
                    for u in range(4):
                        j = 4 * q + u
                        t = g * GRP + j
                        sl = ps[:, u * WP:(u + 1) * WP]
                        if j in DIRECT:
                            di = g * ND + DIRECT.index(j)
                            nc.vector.tensor_reduce(
                                res_d[:, di:di + 1], sl,
                                axis=mybir.AxisListType.X, op=MIN,
                            )
                        else:
                            nc.scalar.copy(xb[:, ASSIST.index(j), :], sl)
                # batched tail: fold 512->64 at fp16 2x, min-reduce 64->1
                v1 = wpool.tile([P, NA, WP // 2], F16, tag="v1")
                nc.vector.tensor_tensor(
                    v1[:], xb[:, :, :WP // 2], xb[:, :, WP // 2:], op=MIN
                )
                v2 = wpool.tile([P, NA, WP // 4], F16, tag="v2")
                nc.vector.tensor_tensor(
                    v2[:], v1[:, :, :WP // 4], v1[:, :, WP // 4:], op=MIN
                )
                v3 = wpool.tile([P, NA, WP // 8], F16, tag="v3")
                nc.vector.tensor_tensor(
                    v3[:], v2[:, :, :WP // 8], v2[:, :, WP // 8:], op=MIN
                )
                nc.vector.tensor_reduce(
                    res_a[:, g * NA:(g + 1) * NA], v3[:],
                    axis=mybir.AxisListType.X, op=MIN,
                )

            nc.sync.dma_start(mind_d.ap(), res_d[:])
            nc.sync.dma_start(mina_d.ap(), res_a[:])

    nc.compile()
    return nc


_NC_CACHE = []


def _get_nc():
    if not _NC_CACHE:
        _NC_CACHE.append(_build_nc())
    return _NC_CACHE[0]


def _prep_side(own, other):
    """Build lhsT [128, N] and rhs [128, N] fp16 with the K=8 row content
    replicated at partition offsets 0/32/64/96 for row-group packing."""
    o16 = own.astype(np.float16)
    t16 = other.astype(np.float16)
    o32 = o16.astype(np.float32)
    t32 = t16.astype(np.float32)
    on = (o32 * o32).sum(-1)       # fp32 norms of the fp16-rounded points
    tn = (t32 * t32).sum(-1)
    on_hi = on.astype(np.float16)
    on_lo = (on - on_hi.astype(np.float32)).astype(np.float16)
    tn_hi = tn.astype(np.float16)
    tn_lo = (tn - tn_hi.astype(np.float32)).astype(np.float16)

    n = own.shape[0]
    lhsT = np.zeros((K, n), np.float16)
    lhsT[0:3] = (-2.0 * o32).astype(np.float16).T
    lhsT[3] = on_hi
    lhsT[4] = on_lo
    lhsT[5] = 1.0
    lhsT[6] = 1.0
    rhs = np.zeros((K, n), np.float16)
    rhs[0:3] = t16.T
    rhs[3] = 1.0
    rhs[4] = 1.0
    rhs[5] = tn_hi
    rhs[6] = tn_lo

    lhsT4 = np.zeros((P, n), np.float16)
    rhs4 = np.zeros((P, n), np.float16)
    for g in range(4):
        lhsT4[32 * g:32 * g + K] = lhsT
        rhs4[32 * g:32 * g + K] = rhs
    return lhsT4, rhs4


def _sides(pred, target):
    """Per-core (own_sorted, other_sorted, axis) for the 8 (batch,
    direction) pairs, with both clouds sorted along the batch's
    max-variance axis."""
    pred = np.asarray(pred, dtype=np.float32)
    target = np.asarray(target, dtype=np.float32)
    sides = []
    for b in range(B):
        axis = int(np.argmax(pred[b].var(0) + target[b].var(0)))
        for d in range(2):
            own, other = (
                (pred[b], target[b]) if d == 0 else (target[b], pred[b])
            )
            so = np.argsort(own[:, axis], kind="stable")
            st = np.argsort(other[:, axis], kind="stable")
            sides.append((own[so], other[st], axis))
    return sides


def _in_maps_for(pred, target):
    in_maps = []
    for own_s, oth_s, _axis in _sides(pred, target):
        lhsT, rhs = _prep_side(own_s, oth_s)
        in_maps.append({"lhsT": lhsT, "rhs": rhs})
    return in_maps


def _assemble_mins(core_res):
    """[NPTS] windowed min per sorted-own point from the two outputs."""
    res_d = core_res["mind"].astype(np.float64)
    res_a = core_res["mina"].astype(np.float64)
    w = np.empty(NPTS)
    for g in range(MT // GRP):
        for j in range(GRP):
            t = g * GRP + j
            if j in DIRECT:
                col = res_d[:, g * ND + DIRECT.index(j)]
            else:
                col = res_a[:, g * NA + ASSIST.index(j)]
            w[t * P:(t + 1) * P] = col
    return w


def kernel(pred, target):
    sides = _sides(pred, target)
    in_maps = []
    for own_s, oth_s, _axis in sides:
        lhsT, rhs = _prep_side(own_s, oth_s)
        in_maps.append({"lhsT": lhsT, "rhs": rhs})
    nc = _get_nc()
    r = bass_utils.run_bass_kernel_spmd(nc, in_maps, core_ids=list(range(8)))

    total = 0.0
    for core_res, (own_s, oth_s, axis) in zip(r.results, sides):
        w = _assemble_mins(core_res)
        # certification: window covers the true NN unless the windowed min
        # exceeds the squared key-distance to the window edge
        okey = oth_s[:, axis].astype(np.float64)
        own_key = own_s[:, axis].astype(np.float64)
        g = np.empty(NPTS)
        for t in range(MT):
            S = _win_start(t)
            ok = own_key[t * P:(t + 1) * P]
            gl = np.inf if S == 0 else ok - okey[S]
            gr = np.inf if S + WP == NPTS else okey[S + WP - 1] - ok
            g[t * P:(t + 1) * P] = np.minimum(gl, gr)
        uncert = np.nonzero(w > 0.98 * g * g)[0]
        if uncert.size:
            d = own_s[uncert, None, :].astype(np.float64) - oth_s[None, :, :]
            w[uncert] = (d * d).sum(-1).min(1)
        total += w.mean()
    return np.array(total / B, dtype=np.float32)


# revision 18
# speedup vs baseline: 1.1287x; 1.1017x over previous
"""Chamfer distance kernel for Trainium2 (8 NeuronCores).

Problem: pred/target [4, 8192, 3] f32 -> scalar
  mean_b( mean_m min_n ||p_bm - q_bn||^2 + mean_n min_m ||p_bm - q_bn||^2 )

Strategy (one "side" per core; 4 batches x 2 directions = 8 cores):
  Each core owns one (batch, direction) pair. Both clouds are sorted on the
  host along one coordinate axis; each m-tile of 128 "own" points only
  computes distances to a static window of WP=512 "other" points centered
  at the matching rank (rank-locality of nearest neighbors in a sorted
  cloud). The result is certified exactly on the host: a point whose
  windowed min distance is smaller than the squared distance from its sort
  key to the window's edge key provably has its true NN inside the window;
  the ~70 uncertified points per cloud get an exact O(N) host recheck.
  This cuts device work 16x versus the full 8192x8192 sweep while keeping
  the result exact up to fp16 rounding of the certified mins.

  Distances are produced on the TensorEngine as K=8 matmuls using the
  identity ||p-q||^2 = -2 p.q + ||p||^2 + ||q||^2:
      lhsT rows: [-2x, -2y, -2z, n_hi, n_lo, 1, 1, 0]   (own points)
      rhs  rows: [ x,   y,  z,  1,    1,  n_hi, n_lo, 0] (other points)
  Inputs are fp16; norms are split hi/lo into two fp16 values so the norm
  contribution keeps ~2^-22 precision; fp16 products are exact in the fp32
  PSUM accumulation. Host prep replicates lhsT/rhs to partition offsets
  0/32/64/96 so four matmuls pack into disjoint 32-row groups via
  tile_position (full-width [128, n] DMAs; narrow [8, n] transfers only
  engage 8 of 128 SBUF lanes and pace the whole pipeline). Input DMA is
  chunked in 1024-column pieces so the first quads start ~1us in. One
  [128, 2048] PSUM tile holds FOUR m-tiles' windows (one bank-aligned
  512-wide out each).

  Drain (the DVE can read at most ONE PSUM operand per instruction):
  3 of every 8 m-tiles are min-reduced straight out of PSUM by the DVE
  (tensor_reduce, fp32); the other 5 are copied to fp16 SBUF by ScalarE
  and folded 512->64 at fp16 2x + min-reduced by the DVE, batched per
  group of 8 m-tiles. This balances ScalarE and VectorE at ~0.45us per
  m-tile each.
"""

import numpy as np

import concourse.bacc as bacc
import concourse.mybir as mybir
import concourse.tile as tile
from concourse import bass_utils

P = 128          # partitions / m-tile size
NPTS = 8192      # points per cloud
B = 4            # batch
K = 8            # matmul contraction (padded)
MT = NPTS // P   # 64 m-tiles
WP = 512         # window columns per m-tile (one PSUM bank)
GRP = 8          # m-tiles per drain-balancing group (2 PSUM quads)
DIRECT = (2, 3, 7)            # j-in-group drained by DVE straight from PSUM
ASSIST = (0, 1, 4, 5, 6)      # j-in-group staged by ScalarE + ladder
ND = len(DIRECT)
NA = len(ASSIST)

F16 = mybir.dt.float16
F32 = mybir.dt.float32
MIN = mybir.AluOpType.min
AXX = None  # set below


def _win_start(t):
    return min(max(t * P + P // 2 - WP // 2, 0), NPTS - WP)


def _build_nc():
    nc = bacc.Bacc(
        "TRN2", target_bir_lowering=False, debug=False, num_devices=8
    )
    lhsT_d = nc.dram_tensor("lhsT", [P, NPTS], F16, kind="ExternalInput")
    rhs_d = nc.dram_tensor("rhs", [P, NPTS], F16, kind="ExternalInput")
    mind_d = nc.dram_tensor("mind", [P, ND * (MT // GRP)], F32,
                            kind="ExternalOutput")
    mina_d = nc.dram_tensor("mina", [P, NA * (MT // GRP)], F32,
                            kind="ExternalOutput")

    with tile.TileContext(nc) as tc:
        with (
            tc.tile_pool(name="const", bufs=1) as const,
            tc.tile_pool(name="psum", bufs=2, space="PSUM") as psum,
            tc.tile_pool(name="xpool", bufs=3) as xpool,
            tc.tile_pool(name="wpool", bufs=2) as wpool,
        ):
            lt4 = const.tile([P, NPTS], F16)
            rt4 = const.tile([P, NPTS], F16)
            res_d = const.tile([P, ND * (MT // GRP)], F32)
            res_a = const.tile([P, NA * (MT // GRP)], F32)
            # full-width chunked input DMA. Each dma_start costs ~640ns of
            # serialized issue time on its sequencer, so: few chunks, sized
            # small-first (fast pipeline start) to large-last, with the two
            # streams issued from different engines (rhs on SP, lhsT on the
            # otherwise-idle GpSimd SWDGE path).
            for c0, w in ((0, 512), (512, 1536), (2048, 2048),
                          (4096, 4096)):
                nc.sync.dma_start(
                    rt4[:, c0:c0 + w], rhs_d.ap()[:, c0:c0 + w]
                )
                nc.sync.dma_start(
                    lt4[:, c0:c0 + w], lhsT_d.ap()[:, c0:c0 + w]
                )

            for g in range(MT // GRP):
                xb = xpool.tile([P, NA, WP], F16, tag="xb")
                for q in range(2):
                    ps = psum.tile([P, 4, WP], F32, tag="ps")
                    for u in range(4):
                        t = g * GRP + 4 * q + u
                        S = _win_start(t)
                        nc.tensor.matmul(
                            ps[:, u, :],
                            lt4[32 * u:32 * u + K, t * P:(t + 1) * P],
                            rt4[32 * u:32 * u + K, S:S + WP],
                            start=True,
                            stop=True,
                            tile_position=(32 * u, 0),
                        )
                    # direct reduces first (decouples DVE from Act), with
                    # adjacent direct m-tiles fused into one [128,2,512] op
                    if q == 0:
                        # quad0: u=2,3 direct (one fused reduce), u=0,1
                        # assist (one 1024-wide copy)
                        nc.vector.tensor_reduce(
                            res_d[:, g * ND:g * ND + 2], ps[:, 2:4, :],
                            axis=mybir.AxisListType.X, op=MIN,
                        )
                        nc.scalar.copy(xb[:, 0:2, :], ps[:, 0:2, :])
                    else:
                        # quad1: u=3 direct, u=0,1,2 assist (one 1024-wide
                        # pair copy + one 512 copy)
                        nc.vector.tensor_reduce(
                            res_d[:, g * ND + 2:g * ND + 3], ps[:, 3, :],
                            axis=mybir.AxisListType.X, op=MIN,
                        )
                        nc.scalar.copy(xb[:, 2:4, :], ps[:, 0:2, :])
                        nc.scalar.copy(xb[:, 4, :], ps[:, 2, :])
                # batched tail: fold 512->64 at fp16 2x, min-reduce 64->1
                v1 = wpool.tile([P, NA, WP // 2], F16, tag="v1")
                nc.vector.tensor_tensor(
                    v1[:], xb[:, :, :WP // 2], xb[:, :, WP // 2:], op=MIN
                )
                v2 = wpool.tile([P, NA, WP // 4], F16, tag="v2")
                nc.vector.tensor_tensor(
                    v2[:], v1[:, :, :WP // 4], v1[:, :, WP // 4:], op=MIN
                )
                v3 = wpool.tile([P, NA, WP // 8], F16, tag="v3")
                nc.vector.tensor_tensor(
                    v3[:], v2[:, :, :WP // 8], v2[:, :, WP // 8:], op=MIN
                )
                nc.vector.tensor_reduce(
                    res_a[:, g * NA:(g + 1) * NA], v3[:],
                    axis=mybir.AxisListType.X, op=MIN,
                )

            nc.sync.dma_start(mind_d.ap(), res_d[:])
            nc.sync.dma_start(mina_d.ap(), res_a[:])

    nc.compile()
    return nc


_NC_CACHE = []


def _get_nc():
    if not _NC_CACHE:
        _NC_CACHE.append(_build_nc())
    return _NC_CACHE[0]


def _prep_side(own, other):
    """Build lhsT [128, N] and rhs [128, N] fp16 with the K=8 row content
    replicated at partition offsets 0/32/64/96 for row-group packing."""
    o16 = own.astype(np.float16)
    t16 = other.astype(np.float16)
    o32 = o16.astype(np.float32)
    t32 = t16.astype(np.float32)
    on = (o32 * o32).sum(-1)       # fp32 norms of the fp16-rounded points
    tn = (t32 * t32).sum(-1)
    on_hi = on.astype(np.float16)
    on_lo = (on - on_hi.astype(np.float32)).astype(np.float16)
    tn_hi = tn.astype(np.float16)
    tn_lo = (tn - tn_hi.astype(np.float32)).astype(np.float16)

    n = own.shape[0]
    lhsT = np.zeros((K, n), np.float16)
    lhsT[0:3] = (-2.0 * o32).astype(np.float16).T
    lhsT[3] = on_hi
    lhsT[4] = on_lo
    lhsT[5] = 1.0
    lhsT[6] = 1.0
    rhs = np.zeros((K, n), np.float16)
    rhs[0:3] = t16.T
    rhs[3] = 1.0
    rhs[4] = 1.0
    rhs[5] = tn_hi
    rhs[6] = tn_lo

    lhsT4 = np.zeros((P, n), np.float16)
    rhs4 = np.zeros((P, n), np.float16)
    for g in range(4):
        lhsT4[32 * g:32 * g + K] = lhsT
        rhs4[32 * g:32 * g + K] = rhs
    return lhsT4, rhs4


def _sides(pred, target):
    """Per-core (own_sorted, other_sorted, axis) for the 8 (batch,
    direction) pairs, with both clouds sorted along the batch's
    max-variance axis."""
    pred = np.asarray(pred, dtype=np.float32)
    target = np.asarray(target, dtype=np.float32)
    sides = []
    for b in range(B):
        axis = int(np.argmax(pred[b].var(0) + target[b].var(0)))
        for d in range(2):
            own, other = (
                (pred[b], target[b]) if d == 0 else (target[b], pred[b])
            )
            so = np.argsort(own[:, axis], kind="stable")
            st = np.argsort(other[:, axis], kind="stable")
            sides.append((own[so], other[st], axis))
    return sides


def _in_maps_for(pred, target):
    in_maps = []
    for own_s, oth_s, _axis in _sides(pred, target):
        lhsT, rhs = _prep_side(own_s, oth_s)
        in_maps.append({"lhsT": lhsT, "rhs": rhs})
    return in_maps


def _assemble_mins(core_res):
    """[NPTS] windowed min per sorted-own point from the two outputs."""
    res_d = core_res["mind"].astype(np.float64)
    res_a = core_res["mina"].astype(np.float64)
    w = np.empty(NPTS)
    for g in range(MT // GRP):
        for j in range(GRP):
            t = g * GRP + j
            if j in DIRECT:
                col = res_d[:, g * ND + DIRECT.index(j)]
            else:
                col = res_a[:, g * NA + ASSIST.index(j)]
            w[t * P:(t + 1) * P] = col
    return w


def kernel(pred, target):
    sides = _sides(pred, target)
    in_maps = []
    for own_s, oth_s, _axis in sides:
        lhsT, rhs = _prep_side(own_s, oth_s)
        in_maps.append({"lhsT": lhsT, "rhs": rhs})
    nc = _get_nc()
    r = bass_utils.run_bass_kernel_spmd(nc, in_maps, core_ids=list(range(8)))

    total = 0.0
    for core_res, (own_s, oth_s, axis) in zip(r.results, sides):
        w = _assemble_mins(core_res)
        # certification: window covers the true NN unless the windowed min
        # exceeds the squared key-distance to the window edge
        okey = oth_s[:, axis].astype(np.float64)
        own_key = own_s[:, axis].astype(np.float64)
        g = np.empty(NPTS)
        for t in range(MT):
            S = _win_start(t)
            ok = own_key[t * P:(t + 1) * P]
            gl = np.inf if S == 0 else ok - okey[S]
            gr = np.inf if S + WP == NPTS else okey[S + WP - 1] - ok
            g[t * P:(t + 1) * P] = np.minimum(gl, gr)
        uncert = np.nonzero(w > 0.98 * g * g)[0]
        if uncert.size:
            d = own_s[uncert, None, :].astype(np.float64) - oth_s[None, :, :]
            w[uncert] = (d * d).sum(-1).min(1)
        total += w.mean()
    return np.array(total / B, dtype=np.float32)
